# revision 27
# baseline (speedup 1.0000x reference)
"""3-layer GCN encoder on 8 TRN2 NeuronCores (Bass/Tile).

Strategy (dst-sharded graph parallel):
  - Each core owns N/8 destination nodes. Per layer:
      dense:  Y' = (dinv*h) @ (W*bn_scale)   [PE, per 128-node tile]
      AllGather Y' rows across cores (bf16)  [TOPSP collective]
      aggregate: for each dst-tile, dma_gather the in-edge source rows
        (edge-major, 256B rows) and reduce on PE with streamed fp8
        0/1 selector matmuls accumulating in PSUM:
           Zt[f,d] += msg_chunk[s,f]^T @ sel_chunk[s,d]
      epilogue: h~ = dinv * lrelu(Z*dinv + bias)  [DVE mul, ACT lrelu]
  - dma_gather indices are int16, so the 50k-row table is addressed
    through two overlapping views (rows [0,32768) and [N-32768, N));
    each dst-tile's slots are segmented into a lo-run and a hi-run.
  - Per-edge normalization dinv[src]*dinv[dst] is folded into row
    scalings (h~ = dinv*h on the way in, *dinv[d] in the epilogue), so
    selectors are pure 0/1 and shared across layers.
"""

import hashlib
import os
import sys

import numpy as np

sys.path.insert(0, "/opt/trn_rl_repo")  # concourse/bass runtime (part of the image)

import ml_dtypes  # noqa: E402

import concourse.bacc as bacc  # noqa: E402
import concourse.mybir as mybir  # noqa: E402
import concourse.tile as tile  # noqa: E402
from concourse.bass_utils import run_bass_kernel_spmd  # noqa: E402
from concourse.tile import add_dep_helper  # noqa: E402

F8 = mybir.dt.float8e4
BF16 = mybir.dt.bfloat16
F32 = mybir.dt.float32
I16 = mybir.dt.int16
NP_F8 = ml_dtypes.float8_e4m3
NP_BF16 = ml_dtypes.bfloat16

GATHER_SEG = 1024          # max idxs per dma_gather (SWDGE ring limit)
NQUEUES = 4                # parallel SWDGE queues


def _ceil_to(x, m):
    return (x + m - 1) // m * m


class Plan:
    """Edge-derived, shared-across-cores instruction plan + per-core data."""

    def __init__(self, src, dst, n_nodes, n_cores, lo_cap=32768):
        N, C = n_nodes, n_cores
        shard = N // C
        assert shard * C == N
        ntiles = _ceil_to(shard, 128) // 128
        self.N, self.C, self.shard, self.ntiles = N, C, shard, ntiles
        self.lo_cap = lo_cap
        self.hi_base = N - lo_cap  # hi view covers rows [hi_base, N)
        assert self.hi_base + lo_cap >= N and (lo_cap <= 32768)

        deg = np.bincount(dst, minlength=N).astype(np.float64) + 1.0
        self.dinv = (1.0 / np.sqrt(deg)).astype(np.float32)

        # per (core, tile): lists of (dst_local, src) split lo/hi
        # build via sort by dst then src-class
        order = np.argsort(dst, kind="stable")
        sdst = dst[order]
        ssrc = src[order]
        # append self edges (src=dst=n)
        alln = np.arange(N, dtype=ssrc.dtype)
        sdst = np.concatenate([sdst, alln])
        ssrc = np.concatenate([ssrc, alln])
        order2 = np.argsort(sdst, kind="stable")
        sdst = sdst[order2]
        ssrc = ssrc[order2]
        starts = np.searchsorted(sdst, np.arange(N + 1))

        is_lo = ssrc < lo_cap

        # raw per-(core,tile) lo/hi slot lists
        lo_lists = [[[] for _ in range(ntiles)] for _ in range(C)]
        hi_lists = [[[] for _ in range(ntiles)] for _ in range(C)]
        for c in range(C):
            for t in range(ntiles):
                d0 = c * shard + t * 128
                d1 = min(c * shard + shard, d0 + 128)
                lo_l = lo_lists[c][t]
                hi_l = hi_lists[c][t]
                for d in range(d0, d1):
                    s0, s1 = starts[d], starts[d + 1]
                    dl = d - d0
                    for k in range(s0, s1):
                        (lo_l if is_lo[k] else hi_l).append((dl, ssrc[k]))
        # common (max-over-cores) chunk counts per tile
        self.lch = [
            _ceil_to(max(len(lo_lists[c][t]) for c in range(C)) or 1, 128) // 128
            for t in range(ntiles)
        ]
        self.hch = [
            _ceil_to(max(max(len(hi_lists[c][t]) for c in range(C)), 0), 128) // 128
            for t in range(ntiles)
        ]
        self.nch = [self.lch[t] + self.hch[t] for t in range(ntiles)]
        self.chunk_base = np.concatenate([[0], np.cumsum(self.nch)]).astype(int)
        self.total_chunks = int(self.chunk_base[-1])
        S = self.total_chunks * 128  # total slots

        # gather segment schedule (shared): per tile, lo segs then hi segs
        # each entry: (tile, seg_slots, slot_off_global, chunk_off_in_tile)
        self.segments = []
        for t in range(ntiles):
            base = int(self.chunk_base[t]) * 128
            for kind, nch_k, coff in (("lo", self.lch[t], 0),
                                      ("hi", self.hch[t], self.lch[t])):
                rem = nch_k * 128
                off = 0
                while rem > 0:
                    seg = min(rem, GATHER_SEG)
                    self.segments.append(
                        (t, kind, seg, base + coff * 128 + off, coff + off // 128)
                    )
                    rem -= seg
                    off += seg

        # per-core data: idx stream + selectors
        self.idx = np.zeros((C, 128, S // 16), np.int16)
        self.sel = np.zeros((C, 128, self.total_chunks, 128), NP_F8)
        one = NP_F8(1.0)
        for c in range(C):
            flat_idx = np.zeros(S, np.int32)
            for t in range(ntiles):
                base = int(self.chunk_base[t]) * 128
                for kind, lst, coff in (("lo", lo_lists[c][t], 0),
                                        ("hi", hi_lists[c][t], self.lch[t])):
                    boff = base + coff * 128
                    for i, (dl, s) in enumerate(lst):
                        flat_idx[boff + i] = s if kind == "lo" else s - self.hi_base
                        q = (boff + i) // 128
                        self.sel[c, (boff + i) % 128, q, dl] = one
            wrapped = flat_idx.astype(np.int16).reshape(S // 16, 16).T
            self.idx[c] = np.tile(wrapped, (8, 1))
        assert (self.idx >= 0).all()


_PLAN_CACHE = {}
_NC_CACHE = {}
_LAST_RES = None


def build_plan(src, dst, n_nodes, n_cores, lo_cap=32768):
    key = (n_nodes, n_cores, lo_cap,
           hashlib.sha1(src.tobytes()).hexdigest(),
           hashlib.sha1(dst.tobytes()).hexdigest())
    p = _PLAN_CACHE.get(key)
    if p is None:
        p = Plan(src, dst, n_nodes, n_cores, lo_cap)
        _PLAN_CACHE.clear()
        _PLAN_CACHE[key] = p
    return p


def build_nc(plan, in_c, hid, out_c):
    """Trace the SPMD program. Returns (nc, meta)."""
    N, C, shard, ntiles = plan.N, plan.C, plan.shard, plan.ntiles
    padded = ntiles * 128
    S = plan.total_chunks * 128
    assert in_c == 128 and hid == 128 and out_c <= 128

    nc = bacc.Bacc("TRN2", target_bir_lowering=False, debug=False,
                   num_devices=C, num_swdge_queues=NQUEUES)

    xT_d = nc.dram_tensor("xT", [128, padded], BF16, kind="ExternalInput").ap()
    W_d = [nc.dram_tensor(f"W{l}", [128, 128], BF16, kind="ExternalInput").ap()
           for l in range(3)]
    biasrow_d = nc.dram_tensor("biasrow", [128, 384], BF16, kind="ExternalInput").ap()
    invdinv_d = nc.dram_tensor("invdinv", [128, padded], BF16, kind="ExternalInput").ap()
    dinv2r_d = nc.dram_tensor("dinv2r", [128, padded], F32, kind="ExternalInput").ap()
    dinv_d = nc.dram_tensor("dinv_rep", [128, shard], F32, kind="ExternalInput").ap()
    sel_d = nc.dram_tensor("sel", [128, plan.total_chunks, 128], F8,
                           kind="ExternalInput").ap()
    idx_d = nc.dram_tensor("idxs", [128, S // 16], I16, kind="ExternalInput").ap()
    out_d = nc.dram_tensor("zT", [out_c, shard], F32, kind="ExternalOutput").ap()

    agin = [nc.dram_tensor(f"agin{l}", [shard, 128], BF16) for l in range(3)]
    agout = [nc.dram_tensor(f"agout{l}", [N, 128], BF16, addr_space="Shared")
             for l in range(3)]
    rg = [list(range(C))]

    max_nch = max(plan.nch)
    qctr = [0]

    with tile.TileContext(nc) as tc:
        with tc.tile_pool(name="const", bufs=1) as constp, \
             tc.tile_pool(name="ht", bufs=2) as htp, \
             tc.tile_pool(name="ysb", bufs=4) as ysbp, \
             tc.tile_pool(name="msg", bufs=3) as msgp, \
             tc.tile_pool(name="sel", bufs=3) as selp, \
             tc.tile_pool(name="epi", bufs=4) as epip, \
             tc.tile_pool(name="zout", bufs=1) as zoutp, \
             tc.tile_pool(name="psy", bufs=4, space="PSUM") as psyp, \
             tc.tile_pool(name="psz", bufs=4, space="PSUM") as pszp:

            # resident constants
            W_s = [constp.tile([128, 128], BF16, tag=f"w{l}", name=f"W_s{l}") for l in range(3)]
            for l in range(3):
                nc.sync.dma_start(out=W_s[l][:], in_=W_d[l][:])
            biasrow_s = constp.tile([128, 384], BF16, tag="biasrow")
            nc.sync.dma_start(out=biasrow_s[:], in_=biasrow_d[:])
            invdinv_s = constp.tile([128, padded], BF16, tag="invdinv")
            nc.sync.dma_start(out=invdinv_s[:], in_=invdinv_d[:])
            dinv2r_s = constp.tile([128, padded], F32, tag="dinv2r")
            nc.sync.dma_start(out=dinv2r_s[:], in_=dinv2r_d[:])
            dinv_s = constp.tile([128, shard], F32, tag="dinv")
            nc.sync.dma_start(out=dinv_s[:], in_=dinv_d[:])
            idx_s = constp.tile([128, S // 16], I16, tag="idx")
            nc.sync.dma_start(out=idx_s[:], in_=idx_d[:])

            hT = htp.tile([128, padded], BF16, tag="ht")
            nc.sync.dma_start(out=hT[:], in_=xT_d[:])

            zT_s = zoutp.tile([out_c, shard], F32, tag="z")

            for l in range(3):
                # ---- dense phase: Y' = hT_j^T @ W  (node-major rows) ----
                agin_dmas = []
                for j in range(ntiles):
                    psy = psyp.tile([128, 128], F32, tag="psy")
                    nc.tensor.matmul(psy[:], hT[:, j * 128:(j + 1) * 128],
                                     W_s[l][:], start=True, stop=True)
                    ysb = ysbp.tile([128, 128], BF16, tag="ysb")
                    nc.scalar.copy(ysb[:], psy[:])
                    r0 = j * 128
                    r1 = min(shard, r0 + 128)
                    agin_dmas.append(nc.sync.dma_start(
                        out=agin[l].ap()[r0:r1], in_=ysb[0:r1 - r0, :]))
                # ---- all-gather Y' rows ----
                cc = nc.gpsimd.collective_compute(
                    "AllGather", mybir.AluOpType.bypass,
                    replica_groups=rg,
                    ins=[agin[l].ap()],
                    outs=[agout[l].ap()],
                )
                for d in agin_dmas:
                    add_dep_helper(cc.ins, d.ins, reason="agin before AG")
                lo_view = agout[l].ap()[0:plan.lo_cap]
                hi_view = agout[l].ap()[plan.hi_base:N]

                # ---- aggregation per dst tile ----
                cc_dep_done = False
                for t in range(ntiles):
                    nch = plan.nch[t]
                    cb = int(plan.chunk_base[t])
                    msg = msgp.tile([128, max_nch, 128], BF16, tag="msg")
                    selt = selp.tile([128, max_nch, 128], F8, tag="sel")
                    nc.sync.dma_start(out=selt[:, 0:nch, :],
                                      in_=sel_d[:, cb:cb + nch, :])
                    for (tt, kind, seg, slot_off, chunk_off) in plan.segments:
                        if tt != t:
                            continue
                        view = lo_view if kind == "lo" else hi_view
                        g = nc.gpsimd.dma_gather(
                            out_ap=msg[:, chunk_off:chunk_off + seg // 128, :],
                            in_ap=view,
                            idxs_ap=idx_s[:, slot_off // 16:(slot_off + seg) // 16],
                            num_idxs=seg, num_idxs_reg=seg, elem_size=128,
                            queue_num=qctr[0] % NQUEUES,
                        )
                        if not cc_dep_done:
                            add_dep_helper(g.ins, cc.ins, reason="AG before gather")
                            cc_dep_done = True
                        qctr[0] += 1
                    psz = pszp.tile([128, 128], F32, tag="psz")
                    for q in range(nch):
                        nc.tensor.matmul(psz[:], msg[:, q, :], selt[:, q, :],
                                         start=(q == 0), stop=False)
                    # rank-1 bias: Z += bias[f] (x) (1/dinv)[d]
                    d0 = t * 128
                    nc.tensor.matmul(
                        psz[:],
                        biasrow_s[0:1, l * 128:(l + 1) * 128],
                        invdinv_s[0:1, d0:d0 + 128],
                        start=False, stop=True)
                    # ---- epilogue: h~ = lrelu(dinv^2 * Z') ----
                    d1 = min(shard, d0 + 128)
                    w = d1 - d0
                    if l < 2:
                        u = epip.tile([128, 128], F32, tag="t1")
                        nc.vector.tensor_tensor(
                            u[:, 0:w], psz[:, 0:w], dinv2r_s[:, d0:d1],
                            op=mybir.AluOpType.mult)
                        nc.vector.scalar_tensor_tensor(
                            hT[:, d0:d1], u[:, 0:w], 0.1, u[:, 0:w],
                            op0=mybir.AluOpType.mult, op1=mybir.AluOpType.max)
                    else:
                        nc.vector.tensor_tensor(
                            zT_s[:, d0:d1], psz[0:out_c, 0:w],
                            dinv_s[0:out_c, d0:d1], op=mybir.AluOpType.mult)
            nc.sync.dma_start(out=out_d[:], in_=zT_s[:])

    nc.compile()
    return nc


def prep_inputs(plan, x, W1, b1, g1, be1, m1, v1, W2, b2, g2, be2, m2, v2,
                W3, b3, out_c):
    """Build the per-core in_maps."""
    EPS = 1e-5
    N, C, shard, ntiles = plan.N, plan.C, plan.shard, plan.ntiles
    padded = ntiles * 128
    dinv = plan.dinv

    def fold(W, b, g, be, m, v):
        s = (g / np.sqrt(v + EPS)).astype(np.float32)
        return (W * s).astype(np.float32), (b * s + be - m * s).astype(np.float32)

    W1f, b1f = fold(W1, b1, g1, be1, m1, v1)
    W2f, b2f = fold(W2, b2, g2, be2, m2, v2)
    W3f = np.zeros((128, 128), np.float32)
    W3f[:, :out_c] = W3
    b3f = np.zeros(128, np.float32)
    b3f[:out_c] = b3

    xt = (x * dinv[:, None]).astype(NP_BF16)  # [N, 128]
    Wb = [W1f.astype(NP_BF16), W2f.astype(NP_BF16), W3f.astype(NP_BF16)]
    biasrow = np.zeros((128, 384), NP_BF16)
    biasrow[0, 0:128] = b1f
    biasrow[0, 128:256] = b2f
    biasrow[0, 256:384] = b3f

    in_maps = []
    for c in range(C):
        xs = xt[c * shard:(c + 1) * shard]  # [shard, 128]
        xT = np.zeros((128, padded), NP_BF16)
        xT[:, :shard] = xs.T
        dl = dinv[c * shard:(c + 1) * shard].astype(np.float64)
        dinv_rep = np.broadcast_to(
            dinv[c * shard:(c + 1) * shard][None, :], (128, shard)
        ).astype(np.float32).copy()
        invd = np.zeros((128, padded), NP_BF16)
        invd[0, :shard] = (1.0 / dl).astype(np.float32)
        d2r = np.zeros((128, padded), np.float32)
        d2r[:, :shard] = (dl * dl).astype(np.float32)[None, :]
        m = {
            "xT": xT,
            "W0": Wb[0], "W1": Wb[1], "W2": Wb[2],
            "biasrow": biasrow,
            "invdinv": invd,
            "dinv2r": d2r,
            "dinv_rep": dinv_rep,
            "sel": plan.sel[c],
            "idxs": plan.idx[c],
        }
        in_maps.append(m)
    return in_maps


def run(plan, nc, in_maps, out_c, trace=False):
    res = run_bass_kernel_spmd(nc, in_maps, list(range(plan.C)), trace=trace)
    shard = plan.shard
    z = np.zeros((plan.N, out_c), np.float32)
    for c in range(plan.C):
        z[c * shard:(c + 1) * shard] = res.results[c]["zT"].T
    return z, res


# ---------------- harness entry point ----------------

N_FULL = 50000
NCORES = 8


def kernel(x, edge_index, W1, b1, g1, be1, m1, v1,
           W2, b2, g2, be2, m2, v2, W3, b3):
    x = np.asarray(x, np.float32)
    ei = np.asarray(edge_index)
    src = np.ascontiguousarray(ei[0], np.int32)
    dst = np.ascontiguousarray(ei[1], np.int32)
    plan = build_plan(src, dst, N_FULL, NCORES)
    key = ("nc", plan.total_chunks)
    nc = _NC_CACHE.get(key)
    if nc is None:
        nc = build_nc(plan, 128, 128, 64)
        _NC_CACHE.clear()
        _NC_CACHE[key] = nc
    in_maps = prep_inputs(plan, x, np.asarray(W1, np.float32), np.asarray(b1, np.float32),
                          np.asarray(g1, np.float32), np.asarray(be1, np.float32),
                          np.asarray(m1, np.float32), np.asarray(v1, np.float32),
                          np.asarray(W2, np.float32), np.asarray(b2, np.float32),
                          np.asarray(g2, np.float32), np.asarray(be2, np.float32),
                          np.asarray(m2, np.float32), np.asarray(v2, np.float32),
                          np.asarray(W3, np.float32), np.asarray(b3, np.float32), 64)
    global _LAST_RES
    z, res = run(plan, nc, in_maps, 64, trace=os.environ.get("GCN_TRACE") == "1")
    _LAST_RES = res
    return z



# revision 29
# speedup vs baseline: 1.0826x; 1.0826x over previous
"""3-layer GCN encoder on 8 TRN2 NeuronCores (Bass/Tile).

Strategy (dst-sharded graph parallel):
  - Each core owns N/8 destination nodes. Per layer:
      dense:  Y' = (dinv*h) @ (W*bn_scale)   [PE, per 128-node tile]
      AllGather Y' rows across cores (bf16)  [TOPSP collective]
      aggregate: for each dst-tile, dma_gather the in-edge source rows
        (edge-major, 256B rows) and reduce on PE with streamed fp8
        0/1 selector matmuls accumulating in PSUM:
           Zt[f,d] += msg_chunk[s,f]^T @ sel_chunk[s,d]
      epilogue: h~ = dinv * lrelu(Z*dinv + bias)  [DVE mul, ACT lrelu]
  - dma_gather indices are int16, so the 50k-row table is addressed
    through two overlapping views (rows [0,32768) and [N-32768, N));
    each dst-tile's slots are segmented into a lo-run and a hi-run.
  - Per-edge normalization dinv[src]*dinv[dst] is folded into row
    scalings (h~ = dinv*h on the way in, *dinv[d] in the epilogue), so
    selectors are pure 0/1 and shared across layers.
"""

import hashlib
import os
import sys

import numpy as np

sys.path.insert(0, "/opt/trn_rl_repo")  # concourse/bass runtime (part of the image)

import ml_dtypes  # noqa: E402

import concourse.bacc as bacc  # noqa: E402
import concourse.mybir as mybir  # noqa: E402
import concourse.tile as tile  # noqa: E402
from concourse.bass_utils import run_bass_kernel_spmd  # noqa: E402
from concourse.tile import add_dep_helper  # noqa: E402

F8 = mybir.dt.float8e4
BF16 = mybir.dt.bfloat16
F32 = mybir.dt.float32
I16 = mybir.dt.int16
NP_F8 = ml_dtypes.float8_e4m3
NP_BF16 = ml_dtypes.bfloat16

GATHER_SEG = 1024          # max idxs per dma_gather (SWDGE ring limit)
NQUEUES = 4                # parallel SWDGE queues


def _ceil_to(x, m):
    return (x + m - 1) // m * m


class Plan:
    """Edge-derived, shared-across-cores instruction plan + per-core data."""

    def __init__(self, src, dst, n_nodes, n_cores, lo_cap=32768):
        N, C = n_nodes, n_cores
        shard = N // C
        assert shard * C == N
        ntiles = _ceil_to(shard, 128) // 128
        self.N, self.C, self.shard, self.ntiles = N, C, shard, ntiles
        self.lo_cap = lo_cap
        self.hi_base = N - lo_cap  # hi view covers rows [hi_base, N)
        assert self.hi_base + lo_cap >= N and (lo_cap <= 32768)

        deg = np.bincount(dst, minlength=N).astype(np.float64) + 1.0
        self.dinv = (1.0 / np.sqrt(deg)).astype(np.float32)

        # per (core, tile): lists of (dst_local, src) split lo/hi
        # build via sort by dst then src-class
        order = np.argsort(dst, kind="stable")
        sdst = dst[order]
        ssrc = src[order]
        # append self edges (src=dst=n)
        alln = np.arange(N, dtype=ssrc.dtype)
        sdst = np.concatenate([sdst, alln])
        ssrc = np.concatenate([ssrc, alln])
        order2 = np.argsort(sdst, kind="stable")
        sdst = sdst[order2]
        ssrc = ssrc[order2]
        starts = np.searchsorted(sdst, np.arange(N + 1))

        is_lo = (ssrc % shard) < 3200  # A-half of each core's shard

        # raw per-(core,tile) lo/hi slot lists
        lo_lists = [[[] for _ in range(ntiles)] for _ in range(C)]
        hi_lists = [[[] for _ in range(ntiles)] for _ in range(C)]
        for c in range(C):
            for t in range(ntiles):
                d0 = c * shard + t * 128
                d1 = min(c * shard + shard, d0 + 128)
                lo_l = lo_lists[c][t]
                hi_l = hi_lists[c][t]
                for d in range(d0, d1):
                    s0, s1 = starts[d], starts[d + 1]
                    dl = d - d0
                    for k in range(s0, s1):
                        (lo_l if is_lo[k] else hi_l).append((dl, ssrc[k]))
        # common (max-over-cores) chunk counts per tile
        self.lch = [
            _ceil_to(max(len(lo_lists[c][t]) for c in range(C)) or 1, 128) // 128
            for t in range(ntiles)
        ]
        self.hch = [
            _ceil_to(max(max(len(hi_lists[c][t]) for c in range(C)), 0), 128) // 128
            for t in range(ntiles)
        ]
        self.nch = [self.lch[t] + self.hch[t] for t in range(ntiles)]
        self.chunk_base = np.concatenate([[0], np.cumsum(self.nch)]).astype(int)
        self.total_chunks = int(self.chunk_base[-1])
        S = self.total_chunks * 128  # total slots

        # gather segment schedule (shared): per tile, lo segs then hi segs
        # each entry: (tile, seg_slots, slot_off_global, chunk_off_in_tile)
        self.segments = []
        for t in range(ntiles):
            base = int(self.chunk_base[t]) * 128
            for kind, nch_k, coff in (("lo", self.lch[t], 0),
                                      ("hi", self.hch[t], self.lch[t])):
                rem = nch_k * 128
                off = 0
                while rem > 0:
                    seg = min(rem, GATHER_SEG)
                    self.segments.append(
                        (t, kind, seg, base + coff * 128 + off, coff + off // 128)
                    )
                    rem -= seg
                    off += seg

        # per-core data: idx stream + selectors
        self.idx = np.zeros((C, 128, S // 16), np.int16)
        self.sel = np.zeros((C, 128, self.total_chunks, 128), NP_F8)
        one = NP_F8(1.0)
        for c in range(C):
            flat_idx = np.zeros(S, np.int32)
            for t in range(ntiles):
                base = int(self.chunk_base[t]) * 128
                for kind, lst, coff in (("lo", lo_lists[c][t], 0),
                                        ("hi", hi_lists[c][t], self.lch[t])):
                    boff = base + coff * 128
                    for i, (dl, s) in enumerate(lst):
                        flat_idx[boff + i] = (
                            (s // shard) * 3200 + (s % shard) if kind == "lo"
                            else (s // shard) * 3050 + (s % shard) - 3200)
                        q = (boff + i) // 128
                        self.sel[c, (boff + i) % 128, q, dl] = one
            wrapped = flat_idx.astype(np.int16).reshape(S // 16, 16).T
            self.idx[c] = np.tile(wrapped, (8, 1))
        assert (self.idx >= 0).all()


_PLAN_CACHE = {}
_NC_CACHE = {}
_LAST_RES = None


def build_plan(src, dst, n_nodes, n_cores, lo_cap=32768):
    key = (n_nodes, n_cores, lo_cap,
           hashlib.sha1(src.tobytes()).hexdigest(),
           hashlib.sha1(dst.tobytes()).hexdigest())
    p = _PLAN_CACHE.get(key)
    if p is None:
        p = Plan(src, dst, n_nodes, n_cores, lo_cap)
        _PLAN_CACHE.clear()
        _PLAN_CACHE[key] = p
    return p


def build_nc(plan, in_c, hid, out_c):
    """Trace the SPMD program. Returns (nc, meta)."""
    N, C, shard, ntiles = plan.N, plan.C, plan.shard, plan.ntiles
    padded = ntiles * 128
    S = plan.total_chunks * 128
    assert in_c == 128 and hid == 128 and out_c <= 128

    nc = bacc.Bacc("TRN2", target_bir_lowering=False, debug=False,
                   num_devices=C, num_swdge_queues=NQUEUES)

    xT_d = nc.dram_tensor("xT", [128, padded], BF16, kind="ExternalInput").ap()
    W_d = [nc.dram_tensor(f"W{l}", [128, 128], BF16, kind="ExternalInput").ap()
           for l in range(3)]
    biasrow_d = nc.dram_tensor("biasrow", [128, 384], BF16, kind="ExternalInput").ap()
    invdinv_d = nc.dram_tensor("invdinv", [128, padded], BF16, kind="ExternalInput").ap()
    dinv2r_d = nc.dram_tensor("dinv2r", [128, padded], F32, kind="ExternalInput").ap()
    dinv_d = nc.dram_tensor("dinv_rep", [128, shard], F32, kind="ExternalInput").ap()
    sel_d = nc.dram_tensor("sel", [128, plan.total_chunks, 128], F8,
                           kind="ExternalInput").ap()
    idx_d = nc.dram_tensor("idxs", [128, S // 16], I16, kind="ExternalInput").ap()
    out_d = nc.dram_tensor("zT", [out_c, shard], F32, kind="ExternalOutput").ap()

    agin = [nc.dram_tensor(f"agin{l}", [shard, 128], BF16) for l in range(3)]
    agoutA = [nc.dram_tensor(f"agoutA{l}", [C * 3200, 128], BF16,
                             addr_space="Shared") for l in range(3)]
    agoutB = [nc.dram_tensor(f"agoutB{l}", [C * 3050, 128], BF16,
                             addr_space="Shared") for l in range(3)]
    rg = [list(range(C))]

    max_nch = max(plan.nch)
    qctr = [0]

    with tile.TileContext(nc) as tc:
        with tc.tile_pool(name="const", bufs=1) as constp, \
             tc.tile_pool(name="ht", bufs=2) as htp, \
             tc.tile_pool(name="ysb", bufs=4) as ysbp, \
             tc.tile_pool(name="msg", bufs=3) as msgp, \
             tc.tile_pool(name="sel", bufs=3) as selp, \
             tc.tile_pool(name="epi", bufs=4) as epip, \
             tc.tile_pool(name="zout", bufs=1) as zoutp, \
             tc.tile_pool(name="psy", bufs=4, space="PSUM") as psyp, \
             tc.tile_pool(name="psz", bufs=4, space="PSUM") as pszp:

            # resident constants
            W_s = [constp.tile([128, 128], BF16, tag=f"w{l}", name=f"W_s{l}") for l in range(3)]
            for l in range(3):
                nc.sync.dma_start(out=W_s[l][:], in_=W_d[l][:])
            biasrow_s = constp.tile([128, 384], BF16, tag="biasrow")
            nc.sync.dma_start(out=biasrow_s[:], in_=biasrow_d[:])
            invdinv_s = constp.tile([128, padded], BF16, tag="invdinv")
            nc.sync.dma_start(out=invdinv_s[:], in_=invdinv_d[:])
            dinv2r_s = constp.tile([128, padded], F32, tag="dinv2r")
            nc.sync.dma_start(out=dinv2r_s[:], in_=dinv2r_d[:])
            dinv_s = constp.tile([128, shard], F32, tag="dinv")
            nc.sync.dma_start(out=dinv_s[:], in_=dinv_d[:])
            idx_s = constp.tile([128, S // 16], I16, tag="idx")
            nc.sync.dma_start(out=idx_s[:], in_=idx_d[:])

            hT = htp.tile([128, padded], BF16, tag="ht")
            nc.sync.dma_start(out=hT[:], in_=xT_d[:])

            zT_s = zoutp.tile([out_c, shard], F32, tag="z")

            def dense_tile(l, j):
                psy = psyp.tile([128, 128], F32, tag="psy", name="psy")
                nc.tensor.matmul(psy[:], hT[:, j * 128:(j + 1) * 128],
                                 W_s[l][:], start=True, stop=True)
                ysb = ysbp.tile([128, 128], BF16, tag="ysb", name="ysb")
                nc.scalar.copy(ysb[:], psy[:])
                r0 = j * 128
                r1 = min(shard, r0 + 128)
                return nc.sync.dma_start(
                    out=agin[l].ap()[r0:r1], in_=ysb[0:r1 - r0, :])

            def make_cc(l, half, dmas):
                ins_ap = (agin[l].ap()[0:3200] if half == "A"
                          else agin[l].ap()[3200:shard])
                out_ap = (agoutA[l].ap() if half == "A" else agoutB[l].ap())
                cc = nc.gpsimd.collective_compute(
                    "AllGather", mybir.AluOpType.bypass,
                    replica_groups=rg, ins=[ins_ap], outs=[out_ap])
                for d in dmas:
                    add_dep_helper(cc.ins, d.ins, reason="agin before AG")
                return cc

            # prologue: dense layer 0; AG-A fires at the halfway tile
            dmasA, dmasB = [], []
            ccA = ccB = None
            for j in range(ntiles):
                (dmasA if j < 25 else dmasB).append(dense_tile(0, j))
                if j == 24:
                    ccA = make_cc(0, "A", dmasA)
            ccB = make_cc(0, "B", dmasB)

            for l in range(3):
                lo_view = agoutA[l].ap()[:]
                hi_view = agoutB[l].ap()[:]
                dmasA, dmasB = [], []
                nextA = None

                # ---- aggregation per dst tile (dense l+1 interleaved) ----
                for t in range(ntiles):
                    nch = plan.nch[t]
                    cb = int(plan.chunk_base[t])
                    msg = msgp.tile([128, max_nch, 128], BF16, tag="msg")
                    selt = selp.tile([128, max_nch, 128], F8, tag="sel")
                    nc.sync.dma_start(out=selt[:, 0:nch, :],
                                      in_=sel_d[:, cb:cb + nch, :])
                    for (tt, kind, seg, slot_off, chunk_off) in plan.segments:
                        if tt != t:
                            continue
                        view = lo_view if kind == "lo" else hi_view
                        ccv = ccA if kind == "lo" else ccB
                        g = nc.gpsimd.dma_gather(
                            out_ap=msg[:, chunk_off:chunk_off + seg // 128, :],
                            in_ap=view,
                            idxs_ap=idx_s[:, slot_off // 16:(slot_off + seg) // 16],
                            num_idxs=seg, num_idxs_reg=seg, elem_size=128,
                            queue_num=qctr[0] % NQUEUES,
                        )
                        add_dep_helper(g.ins, ccv.ins, reason="AG before gather")
                        qctr[0] += 1
                    psz = pszp.tile([128, 128], F32, tag="psz")
                    for q in range(nch):
                        nc.tensor.matmul(psz[:], msg[:, q, :], selt[:, q, :],
                                         start=(q == 0), stop=False)
                    # rank-1 bias: Z += bias[f] (x) (1/dinv)[d]
                    d0 = t * 128
                    nc.tensor.matmul(
                        psz[:],
                        biasrow_s[0:1, l * 128:(l + 1) * 128],
                        invdinv_s[0:1, d0:d0 + 128],
                        start=False, stop=True)
                    # ---- epilogue: h~ = lrelu(dinv^2 * Z') ----
                    d1 = min(shard, d0 + 128)
                    w = d1 - d0
                    if l < 2:
                        u = epip.tile([128, 128], F32, tag="t1")
                        nc.vector.tensor_tensor(
                            u[:, 0:w], psz[:, 0:w], dinv2r_s[:, d0:d1],
                            op=mybir.AluOpType.mult)
                        nc.vector.scalar_tensor_tensor(
                            hT[:, d0:d1], u[:, 0:w], 0.1, u[:, 0:w],
                            op0=mybir.AluOpType.mult, op1=mybir.AluOpType.max)
                        (dmasA if t < 25 else dmasB).append(
                            dense_tile(l + 1, t))
                        if t == 24:
                            nextA = make_cc(l + 1, "A", dmasA)
                    else:
                        nc.vector.tensor_tensor(
                            zT_s[:, d0:d1], psz[0:out_c, 0:w],
                            dinv_s[0:out_c, d0:d1], op=mybir.AluOpType.mult)
                if l < 2:
                    ccA, ccB = nextA, make_cc(l + 1, "B", dmasB)
            nc.sync.dma_start(out=out_d[:], in_=zT_s[:])

    nc.compile()
    return nc


def prep_inputs(plan, x, W1, b1, g1, be1, m1, v1, W2, b2, g2, be2, m2, v2,
                W3, b3, out_c):
    """Build the per-core in_maps."""
    EPS = 1e-5
    N, C, shard, ntiles = plan.N, plan.C, plan.shard, plan.ntiles
    padded = ntiles * 128
    dinv = plan.dinv

    def fold(W, b, g, be, m, v):
        s = (g / np.sqrt(v + EPS)).astype(np.float32)
        return (W * s).astype(np.float32), (b * s + be - m * s).astype(np.float32)

    W1f, b1f = fold(W1, b1, g1, be1, m1, v1)
    W2f, b2f = fold(W2, b2, g2, be2, m2, v2)
    W3f = np.zeros((128, 128), np.float32)
    W3f[:, :out_c] = W3
    b3f = np.zeros(128, np.float32)
    b3f[:out_c] = b3

    xt = (x * dinv[:, None]).astype(NP_BF16)  # [N, 128]
    Wb = [W1f.astype(NP_BF16), W2f.astype(NP_BF16), W3f.astype(NP_BF16)]
    biasrow = np.zeros((128, 384), NP_BF16)
    biasrow[0, 0:128] = b1f
    biasrow[0, 128:256] = b2f
    biasrow[0, 256:384] = b3f

    in_maps = []
    for c in range(C):
        xs = xt[c * shard:(c + 1) * shard]  # [shard, 128]
        xT = np.zeros((128, padded), NP_BF16)
        xT[:, :shard] = xs.T
        dl = dinv[c * shard:(c + 1) * shard].astype(np.float64)
        dinv_rep = np.broadcast_to(
            dinv[c * shard:(c + 1) * shard][None, :], (128, shard)
        ).astype(np.float32).copy()
        invd = np.zeros((128, padded), NP_BF16)
        invd[0, :shard] = (1.0 / dl).astype(np.float32)
        d2r = np.zeros((128, padded), np.float32)
        d2r[:, :shard] = (dl * dl).astype(np.float32)[None, :]
        m = {
            "xT": xT,
            "W0": Wb[0], "W1": Wb[1], "W2": Wb[2],
            "biasrow": biasrow,
            "invdinv": invd,
            "dinv2r": d2r,
            "dinv_rep": dinv_rep,
            "sel": plan.sel[c],
            "idxs": plan.idx[c],
        }
        in_maps.append(m)
    return in_maps


def run(plan, nc, in_maps, out_c, trace=False):
    res = run_bass_kernel_spmd(nc, in_maps, list(range(plan.C)), trace=trace)
    shard = plan.shard
    z = np.zeros((plan.N, out_c), np.float32)
    for c in range(plan.C):
        z[c * shard:(c + 1) * shard] = res.results[c]["zT"].T
    return z, res


# ---------------- harness entry point ----------------

N_FULL = 50000
NCORES = 8


def kernel(x, edge_index, W1, b1, g1, be1, m1, v1,
           W2, b2, g2, be2, m2, v2, W3, b3):
    x = np.asarray(x, np.float32)
    ei = np.asarray(edge_index)
    src = np.ascontiguousarray(ei[0], np.int32)
    dst = np.ascontiguousarray(ei[1], np.int32)
    plan = build_plan(src, dst, N_FULL, NCORES)
    key = ("nc", plan.total_chunks)
    nc = _NC_CACHE.get(key)
    if nc is None:
        nc = build_nc(plan, 128, 128, 64)
        _NC_CACHE.clear()
        _NC_CACHE[key] = nc
    in_maps = prep_inputs(plan, x, np.asarray(W1, np.float32), np.asarray(b1, np.float32),
                          np.asarray(g1, np.float32), np.asarray(be1, np.float32),
                          np.asarray(m1, np.float32), np.asarray(v1, np.float32),
                          np.asarray(W2, np.float32), np.asarray(b2, np.float32),
                          np.asarray(g2, np.float32), np.asarray(be2, np.float32),
                          np.asarray(m2, np.float32), np.asarray(v2, np.float32),
                          np.asarray(W3, np.float32), np.asarray(b3, np.float32), 64)
    global _LAST_RES
    z, res = run(plan, nc, in_maps, 64, trace=os.environ.get("GCN_TRACE") == "1")
    _LAST_RES = res
    return z



# revision 30
# speedup vs baseline: 1.0972x; 1.0135x over previous
"""3-layer GCN encoder on 8 TRN2 NeuronCores (Bass/Tile).

Strategy (dst-sharded graph parallel):
  - Each core owns N/8 destination nodes. Per layer:
      dense:  Y' = (dinv*h) @ (W*bn_scale)   [PE, per 128-node tile]
      AllGather Y' rows across cores (bf16)  [TOPSP collective]
      aggregate: for each dst-tile, dma_gather the in-edge source rows
        (edge-major, 256B rows) and reduce on PE with streamed fp8
        0/1 selector matmuls accumulating in PSUM:
           Zt[f,d] += msg_chunk[s,f]^T @ sel_chunk[s,d]
      epilogue: h~ = dinv * lrelu(Z*dinv + bias)  [DVE mul, ACT lrelu]
  - dma_gather indices are int16, so the 50k-row table is addressed
    through two overlapping views (rows [0,32768) and [N-32768, N));
    each dst-tile's slots are segmented into a lo-run and a hi-run.
  - Per-edge normalization dinv[src]*dinv[dst] is folded into row
    scalings (h~ = dinv*h on the way in, *dinv[d] in the epilogue), so
    selectors are pure 0/1 and shared across layers.
"""

import hashlib
import os
import sys

import numpy as np

sys.path.insert(0, "/opt/trn_rl_repo")  # concourse/bass runtime (part of the image)

import ml_dtypes  # noqa: E402

import concourse.bacc as bacc  # noqa: E402
import concourse.mybir as mybir  # noqa: E402
import concourse.tile as tile  # noqa: E402
from concourse.bass_utils import run_bass_kernel_spmd  # noqa: E402
from concourse.tile import add_dep_helper  # noqa: E402

F8 = mybir.dt.float8e4
BF16 = mybir.dt.bfloat16
F32 = mybir.dt.float32
I16 = mybir.dt.int16
NP_F8 = ml_dtypes.float8_e4m3
NP_BF16 = ml_dtypes.bfloat16

GATHER_SEG = 1024          # max idxs per dma_gather (SWDGE ring limit)
NQUEUES = 4                # parallel SWDGE queues


def _ceil_to(x, m):
    return (x + m - 1) // m * m


class Plan:
    """Edge-derived, shared-across-cores instruction plan + per-core data."""

    def __init__(self, src, dst, n_nodes, n_cores, lo_cap=32768):
        N, C = n_nodes, n_cores
        shard = N // C
        assert shard * C == N
        ntiles = _ceil_to(shard, 128) // 128
        self.N, self.C, self.shard, self.ntiles = N, C, shard, ntiles
        self.lo_cap = lo_cap
        self.hi_base = N - lo_cap  # hi view covers rows [hi_base, N)
        assert self.hi_base + lo_cap >= N and (lo_cap <= 32768)

        deg = np.bincount(dst, minlength=N).astype(np.float64) + 1.0
        self.dinv = (1.0 / np.sqrt(deg)).astype(np.float32)

        # per (core, tile): lists of (dst_local, src) split lo/hi
        # build via sort by dst then src-class
        order = np.argsort(dst, kind="stable")
        sdst = dst[order]
        ssrc = src[order]
        # append self edges (src=dst=n)
        alln = np.arange(N, dtype=ssrc.dtype)
        sdst = np.concatenate([sdst, alln])
        ssrc = np.concatenate([ssrc, alln])
        order2 = np.argsort(sdst, kind="stable")
        sdst = sdst[order2]
        ssrc = ssrc[order2]
        starts = np.searchsorted(sdst, np.arange(N + 1))

        is_lo = (ssrc % shard) < 3200  # A-half of each core's shard

        # raw per-(core,tile) lo/hi slot lists
        lo_lists = [[[] for _ in range(ntiles)] for _ in range(C)]
        hi_lists = [[[] for _ in range(ntiles)] for _ in range(C)]
        for c in range(C):
            for t in range(ntiles):
                d0 = c * shard + t * 128
                d1 = min(c * shard + shard, d0 + 128)
                lo_l = lo_lists[c][t]
                hi_l = hi_lists[c][t]
                for d in range(d0, d1):
                    s0, s1 = starts[d], starts[d + 1]
                    dl = d - d0
                    for k in range(s0, s1):
                        (lo_l if is_lo[k] else hi_l).append((dl, ssrc[k]))
        # common (max-over-cores) chunk counts per tile
        self.lch = [
            _ceil_to(max(len(lo_lists[c][t]) for c in range(C)) or 1, 128) // 128
            for t in range(ntiles)
        ]
        self.hch = [
            _ceil_to(max(max(len(hi_lists[c][t]) for c in range(C)), 0), 128) // 128
            for t in range(ntiles)
        ]
        self.nch = [self.lch[t] + self.hch[t] for t in range(ntiles)]
        self.chunk_base = np.concatenate([[0], np.cumsum(self.nch)]).astype(int)
        self.total_chunks = int(self.chunk_base[-1])
        S = self.total_chunks * 128  # total slots

        # gather segment schedule (shared): per tile, lo segs then hi segs
        # each entry: (tile, seg_slots, slot_off_global, chunk_off_in_tile)
        self.segments = []
        for t in range(ntiles):
            base = int(self.chunk_base[t]) * 128
            for kind, nch_k, coff in (("lo", self.lch[t], 0),
                                      ("hi", self.hch[t], self.lch[t])):
                rem = nch_k * 128
                off = 0
                while rem > 0:
                    seg = min(rem, GATHER_SEG)
                    self.segments.append(
                        (t, kind, seg, base + coff * 128 + off, coff + off // 128)
                    )
                    rem -= seg
                    off += seg

        # per-core data: idx stream + selectors
        self.idx = np.zeros((C, 128, S // 16), np.int16)
        self.sel = np.zeros((C, 128, self.total_chunks, 128), NP_F8)
        one = NP_F8(1.0)
        for c in range(C):
            flat_idx = np.zeros(S, np.int32)
            for t in range(ntiles):
                base = int(self.chunk_base[t]) * 128
                for kind, lst, coff in (("lo", lo_lists[c][t], 0),
                                        ("hi", hi_lists[c][t], self.lch[t])):
                    boff = base + coff * 128
                    for i, (dl, s) in enumerate(lst):
                        flat_idx[boff + i] = (
                            (s // shard) * 3200 + (s % shard) if kind == "lo"
                            else (s // shard) * 3050 + (s % shard) - 3200)
                        q = (boff + i) // 128
                        self.sel[c, (boff + i) % 128, q, dl] = one
            wrapped = flat_idx.astype(np.int16).reshape(S // 16, 16).T
            self.idx[c] = np.tile(wrapped, (8, 1))
        assert (self.idx >= 0).all()


_PLAN_CACHE = {}
_NC_CACHE = {}
_LAST_RES = None


def build_plan(src, dst, n_nodes, n_cores, lo_cap=32768):
    key = (n_nodes, n_cores, lo_cap,
           hashlib.sha1(src.tobytes()).hexdigest(),
           hashlib.sha1(dst.tobytes()).hexdigest())
    p = _PLAN_CACHE.get(key)
    if p is None:
        p = Plan(src, dst, n_nodes, n_cores, lo_cap)
        _PLAN_CACHE.clear()
        _PLAN_CACHE[key] = p
    return p


def build_nc(plan, in_c, hid, out_c):
    """Trace the SPMD program. Returns (nc, meta)."""
    N, C, shard, ntiles = plan.N, plan.C, plan.shard, plan.ntiles
    padded = ntiles * 128
    S = plan.total_chunks * 128
    assert in_c == 128 and hid == 128 and out_c <= 128

    nc = bacc.Bacc("TRN2", target_bir_lowering=False, debug=False,
                   num_devices=C, num_swdge_queues=NQUEUES)

    xT_d = nc.dram_tensor("xT", [128, padded], BF16, kind="ExternalInput").ap()
    W_d = [nc.dram_tensor(f"W{l}", [128, 128], BF16, kind="ExternalInput").ap()
           for l in range(3)]
    biasrow_d = nc.dram_tensor("biasrow", [128, 384], BF16, kind="ExternalInput").ap()
    invdinv_d = nc.dram_tensor("invdinv", [128, padded], BF16, kind="ExternalInput").ap()
    dinv2r_d = nc.dram_tensor("dinv2r", [128, padded], F32, kind="ExternalInput").ap()
    dinv_d = nc.dram_tensor("dinv_rep", [128, shard], F32, kind="ExternalInput").ap()
    sel_d = nc.dram_tensor("sel", [128, plan.total_chunks, 128], F8,
                           kind="ExternalInput").ap()
    idx_d = nc.dram_tensor("idxs", [128, S // 16], I16, kind="ExternalInput").ap()
    out_d = nc.dram_tensor("zT", [out_c, shard], F32, kind="ExternalOutput").ap()

    agin = [nc.dram_tensor(f"agin{l}", [shard, 128], BF16) for l in range(3)]
    agoutA = [nc.dram_tensor(f"agoutA{l}", [C * 3200, 128], BF16,
                             addr_space="Shared") for l in range(3)]
    agoutB = [nc.dram_tensor(f"agoutB{l}", [C * 3050, 128], BF16,
                             addr_space="Shared") for l in range(3)]
    rg = [list(range(C))]

    max_nch = max(plan.nch)
    qctr = [0]

    with tile.TileContext(nc) as tc:
        with tc.tile_pool(name="const", bufs=1) as constp, \
             tc.tile_pool(name="ht", bufs=2) as htp, \
             tc.tile_pool(name="ysb", bufs=4) as ysbp, \
             tc.tile_pool(name="msg", bufs=4) as msgp, \
             tc.tile_pool(name="sel", bufs=4) as selp, \
             tc.tile_pool(name="epi", bufs=4) as epip, \
             tc.tile_pool(name="zout", bufs=1) as zoutp, \
             tc.tile_pool(name="psy", bufs=4, space="PSUM") as psyp, \
             tc.tile_pool(name="psz", bufs=4, space="PSUM") as pszp:

            # resident constants
            W_s = [constp.tile([128, 128], BF16, tag=f"w{l}", name=f"W_s{l}") for l in range(3)]
            for l in range(3):
                nc.sync.dma_start(out=W_s[l][:], in_=W_d[l][:])
            biasrow_s = constp.tile([128, 384], BF16, tag="biasrow")
            nc.sync.dma_start(out=biasrow_s[:], in_=biasrow_d[:])
            invdinv_s = constp.tile([128, padded], BF16, tag="invdinv")
            nc.sync.dma_start(out=invdinv_s[:], in_=invdinv_d[:])
            dinv2r_s = constp.tile([128, padded], F32, tag="dinv2r")
            nc.sync.dma_start(out=dinv2r_s[:], in_=dinv2r_d[:])
            dinv_s = constp.tile([128, shard], F32, tag="dinv")
            nc.sync.dma_start(out=dinv_s[:], in_=dinv_d[:])
            idx_s = constp.tile([128, S // 16], I16, tag="idx")
            nc.sync.dma_start(out=idx_s[:], in_=idx_d[:])

            hT = htp.tile([128, padded], BF16, tag="ht")
            nc.sync.dma_start(out=hT[:], in_=xT_d[:])

            zT_s = zoutp.tile([out_c, shard], F32, tag="z")

            def dense_tile(l, j):
                psy = psyp.tile([128, 128], F32, tag="psy", name="psy")
                nc.tensor.matmul(psy[:], hT[:, j * 128:(j + 1) * 128],
                                 W_s[l][:], start=True, stop=True)
                ysb = ysbp.tile([128, 128], BF16, tag="ysb", name="ysb")
                nc.scalar.copy(ysb[:], psy[:])
                r0 = j * 128
                r1 = min(shard, r0 + 128)
                return nc.sync.dma_start(
                    out=agin[l].ap()[r0:r1], in_=ysb[0:r1 - r0, :])

            def make_cc(l, half, dmas):
                ins_ap = (agin[l].ap()[0:3200] if half == "A"
                          else agin[l].ap()[3200:shard])
                out_ap = (agoutA[l].ap() if half == "A" else agoutB[l].ap())
                cc = nc.gpsimd.collective_compute(
                    "AllGather", mybir.AluOpType.bypass,
                    replica_groups=rg, ins=[ins_ap], outs=[out_ap])
                for d in dmas:
                    add_dep_helper(cc.ins, d.ins, reason="agin before AG")
                return cc

            # prologue: dense layer 0; AG-A fires at the halfway tile
            dmasA, dmasB = [], []
            ccA = ccB = None
            for j in range(ntiles):
                (dmasA if j < 25 else dmasB).append(dense_tile(0, j))
                if j == 24:
                    ccA = make_cc(0, "A", dmasA)
            ccB = make_cc(0, "B", dmasB)

            for l in range(3):
                lo_view = agoutA[l].ap()[:]
                hi_view = agoutB[l].ap()[:]
                dmasA, dmasB = [], []
                nextA = None

                # ---- aggregation per dst tile (dense l+1 interleaved) ----
                for t in range(ntiles):
                    nch = plan.nch[t]
                    cb = int(plan.chunk_base[t])
                    msg = msgp.tile([128, max_nch, 128], BF16, tag="msg")
                    selt = selp.tile([128, max_nch, 128], F8, tag="sel")
                    nc.sync.dma_start(out=selt[:, 0:nch, :],
                                      in_=sel_d[:, cb:cb + nch, :])
                    for (tt, kind, seg, slot_off, chunk_off) in plan.segments:
                        if tt != t:
                            continue
                        view = lo_view if kind == "lo" else hi_view
                        ccv = ccA if kind == "lo" else ccB
                        g = nc.gpsimd.dma_gather(
                            out_ap=msg[:, chunk_off:chunk_off + seg // 128, :],
                            in_ap=view,
                            idxs_ap=idx_s[:, slot_off // 16:(slot_off + seg) // 16],
                            num_idxs=seg, num_idxs_reg=seg, elem_size=128,
                            queue_num=qctr[0] % NQUEUES,
                        )
                        add_dep_helper(g.ins, ccv.ins, reason="AG before gather")
                        qctr[0] += 1
                    psz = pszp.tile([128, 128], F32, tag="psz")
                    for q in range(nch):
                        nc.tensor.matmul(psz[:], msg[:, q, :], selt[:, q, :],
                                         start=(q == 0), stop=False)
                    # rank-1 bias: Z += bias[f] (x) (1/dinv)[d]
                    d0 = t * 128
                    nc.tensor.matmul(
                        psz[:],
                        biasrow_s[0:1, l * 128:(l + 1) * 128],
                        invdinv_s[0:1, d0:d0 + 128],
                        start=False, stop=True)
                    # ---- epilogue: h~ = lrelu(dinv^2 * Z') ----
                    d1 = min(shard, d0 + 128)
                    w = d1 - d0
                    if l < 2:
                        u = epip.tile([128, 128], F32, tag="t1")
                        nc.vector.tensor_tensor(
                            u[:, 0:w], psz[:, 0:w], dinv2r_s[:, d0:d1],
                            op=mybir.AluOpType.mult)
                        nc.vector.scalar_tensor_tensor(
                            hT[:, d0:d1], u[:, 0:w], 0.1, u[:, 0:w],
                            op0=mybir.AluOpType.mult, op1=mybir.AluOpType.max)
                        (dmasA if t < 25 else dmasB).append(
                            dense_tile(l + 1, t))
                        if t == 24:
                            nextA = make_cc(l + 1, "A", dmasA)
                    else:
                        nc.vector.tensor_tensor(
                            zT_s[:, d0:d1], psz[0:out_c, 0:w],
                            dinv_s[0:out_c, d0:d1], op=mybir.AluOpType.mult)
                if l < 2:
                    ccA, ccB = nextA, make_cc(l + 1, "B", dmasB)
            nc.sync.dma_start(out=out_d[:], in_=zT_s[:])

    nc.compile()
    return nc


def prep_inputs(plan, x, W1, b1, g1, be1, m1, v1, W2, b2, g2, be2, m2, v2,
                W3, b3, out_c):
    """Build the per-core in_maps."""
    EPS = 1e-5
    N, C, shard, ntiles = plan.N, plan.C, plan.shard, plan.ntiles
    padded = ntiles * 128
    dinv = plan.dinv

    def fold(W, b, g, be, m, v):
        s = (g / np.sqrt(v + EPS)).astype(np.float32)
        return (W * s).astype(np.float32), (b * s + be - m * s).astype(np.float32)

    W1f, b1f = fold(W1, b1, g1, be1, m1, v1)
    W2f, b2f = fold(W2, b2, g2, be2, m2, v2)
    W3f = np.zeros((128, 128), np.float32)
    W3f[:, :out_c] = W3
    b3f = np.zeros(128, np.float32)
    b3f[:out_c] = b3

    xt = (x * dinv[:, None]).astype(NP_BF16)  # [N, 128]
    Wb = [W1f.astype(NP_BF16), W2f.astype(NP_BF16), W3f.astype(NP_BF16)]
    biasrow = np.zeros((128, 384), NP_BF16)
    biasrow[0, 0:128] = b1f
    biasrow[0, 128:256] = b2f
    biasrow[0, 256:384] = b3f

    in_maps = []
    for c in range(C):
        xs = xt[c * shard:(c + 1) * shard]  # [shard, 128]
        xT = np.zeros((128, padded), NP_BF16)
        xT[:, :shard] = xs.T
        dl = dinv[c * shard:(c + 1) * shard].astype(np.float64)
        dinv_rep = np.broadcast_to(
            dinv[c * shard:(c + 1) * shard][None, :], (128, shard)
        ).astype(np.float32).copy()
        invd = np.zeros((128, padded), NP_BF16)
        invd[0, :shard] = (1.0 / dl).astype(np.float32)
        d2r = np.zeros((128, padded), np.float32)
        d2r[:, :shard] = (dl * dl).astype(np.float32)[None, :]
        m = {
            "xT": xT,
            "W0": Wb[0], "W1": Wb[1], "W2": Wb[2],
            "biasrow": biasrow,
            "invdinv": invd,
            "dinv2r": d2r,
            "dinv_rep": dinv_rep,
            "sel": plan.sel[c],
            "idxs": plan.idx[c],
        }
        in_maps.append(m)
    return in_maps


def run(plan, nc, in_maps, out_c, trace=False):
    res = run_bass_kernel_spmd(nc, in_maps, list(range(plan.C)), trace=trace)
    shard = plan.shard
    z = np.zeros((plan.N, out_c), np.float32)
    for c in range(plan.C):
        z[c * shard:(c + 1) * shard] = res.results[c]["zT"].T
    return z, res


# ---------------- harness entry point ----------------

N_FULL = 50000
NCORES = 8


def kernel(x, edge_index, W1, b1, g1, be1, m1, v1,
           W2, b2, g2, be2, m2, v2, W3, b3):
    x = np.asarray(x, np.float32)
    ei = np.asarray(edge_index)
    src = np.ascontiguousarray(ei[0], np.int32)
    dst = np.ascontiguousarray(ei[1], np.int32)
    plan = build_plan(src, dst, N_FULL, NCORES)
    key = ("nc", plan.total_chunks)
    nc = _NC_CACHE.get(key)
    if nc is None:
        nc = build_nc(plan, 128, 128, 64)
        _NC_CACHE.clear()
        _NC_CACHE[key] = nc
    in_maps = prep_inputs(plan, x, np.asarray(W1, np.float32), np.asarray(b1, np.float32),
                          np.asarray(g1, np.float32), np.asarray(be1, np.float32),
                          np.asarray(m1, np.float32), np.asarray(v1, np.float32),
                          np.asarray(W2, np.float32), np.asarray(b2, np.float32),
                          np.asarray(g2, np.float32), np.asarray(be2, np.float32),
                          np.asarray(m2, np.float32), np.asarray(v2, np.float32),
                          np.asarray(W3, np.float32), np.asarray(b3, np.float32), 64)
    global _LAST_RES
    z, res = run(plan, nc, in_maps, 64, trace=os.environ.get("GCN_TRACE") == "1")
    _LAST_RES = res
    return z

